# revision 37
# baseline (speedup 1.0000x reference)
"""Trainium2 Bass kernel for nn_AdjGenerator (gnn_message_passing), v2.

Reference computation (N=M=2048, D=1024, EMB=64, H=16):
    pos    = relu(einsum('he,benm->bhnm', Wg_w, position_embedding) + Wg_b)
    aff    = (ref_feat @ Wq_w.T + Wq_b) @ (sup_feat @ Wk_w.T + Wk_b).T / sqrt(D)
    sig    = sigmoid(log(pos + 1e-6) + aff[:, None, :]).mean(heads)
    adj    = where(sig > 0.5, 1.0, 0.0)

Same restructurings as v1 (see kernel_v1.py): sigmoid/log folded into
S = E * sum_h 1/(relu(conv_h) + eps + E) with E = exp(-aff); weights folded
into G = Wq_w.T @ Wk_w on the host; position projection runs on TensorE with
a block-diagonal [128, 32] x4 col-tiled weight layout packing 8 n-rows x 16
heads per PSUM tile.

v2 changes (v1 was TensorE-bound: fp32 matmuls run at 4 cycles/row and the
conv / E-broadcast / head-sum matmuls alone cost ~655 us/core):
  * The conv runs in bf16 (1 cycle/row) over a host-cast bf16 pe stream,
    halving the dominant HBM read from 128 to 64 MiB/core; the E-broadcast
    and head-sum matmuls take their moving operands as float32r (1 cycle/
    row, ~12-bit mantissa).  PSUM accumulation stays f32 and the affinity
    path (zT/aff/E) stays full f32.
  * The device returns the f32 score S instead of thresholding; the host
    computes adj = S < 8 and re-evaluates exactly (f64) the entries with
    |S - 8| < TAU, which covers every entry reduced precision could
    misclassify (measured max gate-path |S_dev - S_true| ~0.08, plus the
    f32r rounding, against TAU = 0.25).
  * DMA/engine-queue shaping for this (virtualized) target, where per-op
    and per-DMA overheads are ~usec-scale: the pe stream is a host-
    pre-tiled image read as 512 KiB uint32-element pieces (64 partitions x
    8 KiB lines, 16 KiB source chunks - measured 330 GB/s/core vs 29 for a
    contiguous bf16-element read); phase-1 operands load as a few big
    resident images; engine queues are strict FIFO so the gate visits
    PE -> Act -> DVE once per iteration and the PE head-sum consuming the
    chain is emitted RT_LAG iterations late; matmul start=True clears the
    whole PSUM bank, so concurrent accumulation groups get exclusive
    banks.

Sharding: rows of ref_feat / N axis of position_embedding split across the
8 cores; sup_feat and all weights replicated.  No collectives.
"""

import math

import numpy as np
import ml_dtypes

import concourse.bass as bass
import concourse.mybir as mybir
import concourse.tile as tile
from concourse import bacc

F32 = mybir.dt.float32
BF16 = mybir.dt.bfloat16
NPBF16 = ml_dtypes.bfloat16
AF = mybir.ActivationFunctionType
ALU = mybir.AluOpType

EMB = 64
HEADS = 16
NCORES = 8
EPS = 1e-6
TAU = 0.25


def build_module(ns, m, dc, scale, pe_bufs=6, passes=1,
                 stages=("p1", "conv", "gate", "eprt")):
    """Emit the per-core Bass module.

    ns: rows of ref_feat handled by this core (multiple of 128)
    m:  number of sup rows / columns of S (multiple of 512)
    dc: number of 128-wide contraction chunks of the feature dim
    scale: 1/sqrt(D) applied inside the exp
    passes: emit the whole computation this many times (timing-slope runs)
    stages: subset of {p1, conv, gate, eprt} for HW bisection probes
    """
    d = dc * 128
    NB = ns // 128
    MS = m // 512
    assert ns % 128 == 0 and m % 512 == 0

    nc = bacc.Bacc("TRN2", target_bir_lowering=False, debug=False)
    # pe layout [e, nb, t, cc, s, qq, m] viewed as uint32 pairs: per DMA
    # piece (h, cc) the source is 32 e-chunks of 16 KiB contiguous (strided
    # ~1 MiB) landing on 64 partitions x 8 KiB lines.  This exact shape
    # measured 330 GB/s/core; the same bytes as one contiguous 2 MiB
    # bf16-element read measured 29 GB/s (the DMA path here is largely
    # element-rate-bound, so 4-byte elements double throughput over bf16).
    pe_h = nc.dram_tensor("pe", [EMB, NB, 16, 2, 2, 2, m // 2],
                          mybir.dt.uint32, kind="ExternalInput")
    # phase-1 operands as pre-arranged SBUF images (one or two big DMAs
    # each instead of dozens of small latency-bound loads): col k*W + c
    # holds row k*128+p, col c of the logical matrix
    refT_h = nc.dram_tensor("refT", [128, dc * ns], mybir.dt.float32r,
                            kind="ExternalInput")
    supT_h = nc.dram_tensor("supT", [m // 512, 128, dc * 512],
                            mybir.dt.float32r, kind="ExternalInput")
    G_h = nc.dram_tensor("G", [128, dc * d], mybir.dt.float32r,
                         kind="ExternalInput")
    w2_h = nc.dram_tensor("W2", [128, 32], BF16, kind="ExternalInput")
    dup_h = nc.dram_tensor("dupM", [128, 16 * 128], mybir.dt.float32r,
                           kind="ExternalInput")
    e32_h = nc.dram_tensor("E32", [128, 16 * 128], mybir.dt.float32r,
                           kind="ExternalInput")
    br_h = nc.dram_tensor("brep", [128, 1], F32, kind="ExternalInput")
    s_h = nc.dram_tensor("S", [ns, m], F32, kind="ExternalOutput")

    # All SBUF pools live for the whole kernel (no releases): SBUF addresses
    # are then never reused, so no cross-phase WAR hazards can exist and the
    # pe-stream DMAs are free to run arbitrarily far ahead of phase 1.
    F32R = mybir.dt.float32r
    with tile.TileContext(nc) as tc, \
            tc.tile_pool(name="consts", bufs=1) as cpool, \
            tc.tile_pool(name="persist", bufs=1) as ppool, \
            tc.tile_pool(name="pest", bufs=pe_bufs) as pepool, \
            tc.tile_pool(name="p1", bufs=1) as p1, \
            tc.tile_pool(name="gst", bufs=1) as g_pool, \
            tc.tile_pool(name="supp", bufs=2) as sup_pool, \
            tc.tile_pool(name="work", bufs=2) as wk_pool, \
            tc.tile_pool(name="wkr", bufs=4) as wkr_pool, \
            tc.tile_pool(name="outp", bufs=2) as out_pool:

        w2 = cpool.tile([128, 32], BF16, tag="w2")
        nc.sync.dma_start(out=w2, in_=w2_h[:, :])
        dup = cpool.tile([128, 2048], F32R, tag="dup")
        nc.sync.dma_start(out=dup, in_=dup_h[:, :])
        e32 = cpool.tile([128, 2048], F32R, tag="e32")
        nc.sync.dma_start(out=e32, in_=e32_h[:, :])
        br = cpool.tile([128, 1], F32, tag="br")
        nc.sync.dma_start(out=br, in_=br_h[:, :])

        E_sb = [ppool.tile([128, m], F32, tag=f"E{nb}", name=f"E{nb}")
                for nb in range(NB)]
        Ep_sb = [ppool.tile([128, m], F32R, tag=f"Ep{nb}", name=f"Ep{nb}")
                 for nb in range(NB)]
        zT_sb = [ppool.tile([128, ns], F32R, tag=f"zT{k}",
                        name=f"zT{k}") for k in range(dc)]

        for _pass in range(passes):
            _phases(nc, tc, p1, g_pool, sup_pool, (wk_pool, wkr_pool),
                    out_pool, pepool, pe_h, refT_h, supT_h, G_h, s_h,
                    w2, dup, e32, br, E_sb, Ep_sb, zT_sb,
                    ns, m, dc, NB, MS, scale, set(stages))
    nc.compile()  # bacc legalization: TRN2 allows max 1 sync wait per inst
    return nc


def _phases(nc, tc, p1, g_pool, sup_pool, wk_pools, out_pool, pepool,
            pe_h, refT_h, supT_h, G_h, s_h,
            w2, dup, e32, br, E_sb, Ep_sb, zT_sb,
            ns, m, dc, NB, MS, scale, stages):
    wk_pool, wkr_pool = wk_pools
    F32R = mybir.dt.float32r
    # ---------- phase 1 (f32): zT = G.T @ refT; aff = zT.T @ supT;
    # E = exp(-scale*aff); Ep = f32r(E + eps).
    # G streams through 2 slots and sup tiles have their own slots, so no
    # phase-1 DMA ever waits on compute: the pe-stream DMAs queued behind
    # them on the same HWDGE queues would inherit any such stall.
    with tc.tile_pool(name="zps", bufs=1, space="PSUM") as zps, \
            tc.tile_pool(name="aps", bufs=2, space="PSUM") as aps:
      if "p1" in stages:
        # G and refT as resident SBUF images loaded by two partition-half
        # DMAs each; sup in one image per 512-column block.  Batching these
        # (vs dozens of per-chunk loads) keeps the latency-bound DMA count
        # small and leaves the queues free for the pe stream.
        ref_sb = p1.tile([128, dc * ns], F32R, tag="ref", name="ref")
        nc.sync.dma_start(out=ref_sb[0:64, :], in_=refT_h[0:64, :])
        nc.scalar.dma_start(out=ref_sb[64:128, :], in_=refT_h[64:128, :])
        # 512 KiB pieces with 8 KiB lines (measured-fast DMA shape); a
        # single contiguous 2 MiB half measured ~29 GB/s and would stall
        # the pe stream queued behind it
        g_sb = g_pool.tile([128, dc * dc * 128], F32R, tag="g", name="g")
        gW = dc * dc * 128
        for h in range(2):
            for cq in range(0, gW, 2048):
                eng = nc.sync if h == 0 else nc.scalar
                eng.dma_start(out=g_sb[64 * h:64 * h + 64,
                                       cq:cq + min(2048, gW - cq)],
                              in_=G_h[64 * h:64 * h + 64,
                                      cq:cq + min(2048, gW - cq)])
        # two column-waves of <=4 zT chunks: matmul start=True clears the
        # whole PSUM bank, so every open accumulation group needs its own
        # bank (4 zp banks + 2 aff banks fit; all 8 chunks at once do not)
        for i0 in range(0, dc, 4):
            iw = min(4, dc - i0)
            zp = [zps.tile([128, ns], F32, tag=f"zp{i4}", name=f"zp{i0+i4}")
                  for i4 in range(iw)]
            for k in range(dc):
                for i4 in range(iw):
                    nc.tensor.matmul(
                        zp[i4],
                        g_sb[:, k * dc * 128 + (i0 + i4) * 128:
                             k * dc * 128 + (i0 + i4 + 1) * 128],
                        ref_sb[:, k * ns:(k + 1) * ns],
                        start=(k == 0), stop=(k == dc - 1))
            for i4 in range(iw):
                nc.vector.tensor_scalar_add(zT_sb[i0 + i4], zp[i4], 0.0)
        for msi in range(MS):
            sup_sb = sup_pool.tile([128, dc * 512], F32R, tag="sup",
                                   name=f"sup{msi}")
            sW = dc * 512
            for h in range(2):
                for cq in range(0, sW, 2048):
                    eng = nc.sync if h == 0 else nc.scalar
                    eng.dma_start(
                        out=sup_sb[64 * h:64 * h + 64,
                                   cq:cq + min(2048, sW - cq)],
                        in_=supT_h[msi, 64 * h:64 * h + 64,
                                   cq:cq + min(2048, sW - cq)])
            for nb in range(NB):
                ap_ = aps.tile([128, 512], F32, tag="ap", name="ap")
                for k in range(dc):
                    nc.tensor.matmul(
                        ap_,
                        zT_sb[k][:, nb * 128:(nb + 1) * 128],
                        sup_sb[:, k * 512:(k + 1) * 512],
                        start=(k == 0), stop=(k == dc - 1))
                msl = slice(msi * 512, (msi + 1) * 512)
                nc.scalar.activation(E_sb[nb][:, msl], ap_, AF.Exp,
                                     scale=-scale)
                nc.vector.tensor_scalar_add(
                    Ep_sb[nb][:, msl], E_sb[nb][:, msl], EPS)

    # ---------- phase 2: stream bf16 pe, gate, reduce heads, emit S
    #
    # Engine queues are strict FIFO and a cross-engine dependency chain
    # emitted per iteration serializes every hop (measured +1.4 ms), so the
    # per-iteration chain visits each engine once in a fixed direction
    # (PE conv/ep -> Act relu -> DVE add+recip -> PE Rt) and the only
    # backward edge, the Rt matmul, is emitted RT_LAG iterations late so PE
    # never waits on DVE.  The head-sum and E-broadcast matmuls take their
    # moving operands as float32r (1 cycle/row, ~12-bit mantissa - the
    # rounding is covered by the host refinement band) which removes any
    # f32->bf16 cast from the chain.
    RT_LAG = 2
    with tc.tile_pool(name="wp", bufs=2, space="PSUM") as wp_pool, \
            tc.tile_pool(name="epp", bufs=2, space="PSUM") as ep_pool, \
            tc.tile_pool(name="rp", bufs=MS, space="PSUM") as r_pool:
        for nb in range(NB):
            Rt = [r_pool.tile([128, 512], F32, tag="R", name=f"R{nb}_{mb}")
                  for mb in range(MS)]
            pend = []          # records (t, mb, r_sb)
            for t in range(16):
                pt = pepool.tile([128, 4 * m], BF16, tag="pe")
                for h in range(2):
                    for cc in range(2):
                        # dst [64, m] u32 <- src [32, 2, 2, m/2] u32: the
                        # DMA pairs both sides flat row-major, so partition
                        # 2e+s col qq*m+mc reads src (e, s, qq, mc).
                        eng = nc.sync if h == 0 else nc.scalar
                        eng.dma_start(
                            out=pt[64 * h:64 * h + 64,
                                   cc * 2 * m:(cc + 1) * 2 * m].bitcast(
                                       mybir.dt.uint32),
                            in_=pe_h[32 * h:32 * h + 32, nb, t, cc])
                for mb in range(MS):
                    mbl = slice(mb * 512, (mb + 1) * 512)
                    wp = wp_pool.tile([128, 512], F32, tag="w")
                    # 4 col-tiled matmuls, each filling a disjoint
                    # 32-partition range; moving slices of the packed tile
                    if "conv" in stages:
                        for q2 in range(4):
                            nc.tensor.matmul(
                                wp[q2 * 32:(q2 + 1) * 32, :], w2,
                                pt[:, q2 * m + mb * 512:
                                   q2 * m + (mb + 1) * 512],
                                start=True, stop=True,
                                tile_position=(0, q2 * 32))
                    if "eprt" in stages:
                        ep = ep_pool.tile([128, 512], F32, tag="ep")
                        nc.tensor.matmul(ep, dup[:, t * 128:(t + 1) * 128],
                                         Ep_sb[nb][:, mbl],
                                         start=True, stop=True)
                    if "gate" in stages:
                        w_sb = wk_pool.tile([128, 512], F32, tag="w_sb")
                        nc.scalar.activation(w_sb, wp, AF.Relu,
                                             bias=br[:, 0:1])
                        d_sb = wk_pool.tile([128, 512], F32, tag="d_sb")
                        if "eprt" in stages:
                            nc.vector.tensor_add(d_sb, w_sb, ep)
                        else:
                            nc.vector.tensor_scalar_add(d_sb, w_sb, 1.0)
                        r_sb = wkr_pool.tile([128, 512], F32R, tag="r_sb")
                        # reciprocal_approx_fast with an f32r output (the
                        # wrapper asserts f32/f32; the op's bit tricks are on
                        # the INPUT, and the f32r write rounds on writeback,
                        # which the BIR verifier requires for f32r consumers)
                        from concourse.dve_ops import (
                            RECIP_APPROX_FAST_CONSTS,
                            RECIPROCAL_APPROX_FAST,
                        )
                        _c = RECIP_APPROX_FAST_CONSTS
                        nc.vector._custom_dve(
                            RECIPROCAL_APPROX_FAST, out=r_sb, in0=d_sb,
                            s0=_c["s0"], s1=_c["s1"], imm2=_c["imm2"])
                        if "eprt" in stages:
                            pend.append((t, mb, r_sb))
                            if len(pend) > RT_LAG:
                                t0, mb0, r0 = pend.pop(0)
                                nc.tensor.matmul(
                                    Rt[mb0],
                                    e32[:, t0 * 128:(t0 + 1) * 128], r0,
                                    start=(t0 == 0), stop=(t0 == 15))
                    elif "eprt" in stages:
                        nc.tensor.matmul(Rt[mb],
                                         e32[:, t * 128:(t + 1) * 128],
                                         Ep_sb[nb][:, mbl],
                                         start=(t == 0), stop=(t == 15))
            if "gate" in stages and "eprt" in stages:
                for t0, mb0, r0 in pend:
                    nc.tensor.matmul(Rt[mb0],
                                     e32[:, t0 * 128:(t0 + 1) * 128], r0,
                                     start=(t0 == 0), stop=(t0 == 15))
                pend.clear()
            if "p1" in stages and "eprt" in stages:
                for mb in range(MS):
                    mbl = slice(mb * 512, (mb + 1) * 512)
                    tm = out_pool.tile([128, 512], F32, tag="tm")
                    nc.vector.tensor_mul(tm, E_sb[nb][:, mbl], Rt[mb])
                    eng = nc.sync if mb % 2 == 0 else nc.scalar
                    eng.dma_start(out=s_h[nb * 128:(nb + 1) * 128, mbl],
                                  in_=tm)


def make_consts(Wg_w, Wg_b):
    """Constant 0/1 routing matrices + the packed projection weights."""
    Wg_w = np.asarray(Wg_w, np.float32)
    Wg_b = np.asarray(Wg_b, np.float32)
    assert Wg_w.shape == (HEADS, EMB)
    # shared col-group weight: rows (2e + s), cols (16*s + h) = Wg_w[h, e]
    # (row layout matches the host pe tiling: partition p = 2e + s)
    W2 = np.zeros((128, 32), np.float32)
    for s in range(2):
        W2[s::2, 16 * s:16 * s + 16] = Wg_w.T
    dupM = np.zeros((128, 2048), np.float32)
    E32 = np.zeros((128, 2048), np.float32)
    for t in range(16):
        for p in range(128):
            dupM[8 * t + (p >> 4), t * 128 + p] = 1.0
            E32[p, t * 128 + 8 * t + (p >> 4)] = 1.0
    brep = Wg_b[np.arange(128) % 16].reshape(128, 1).astype(np.float32)
    return W2, dupM, E32, brep


def make_gmat(Wq_w, Wq_b, Wk_w, Wk_b):
    """Fold q/k projections into one matrix: aff_core = refA @ G @ supA.T.

    With nonzero biases, G gains an extra row/col pairing with the ones
    column appended to ref/sup on the host (padded to a 128 multiple).
    Returns (G, d_pad, use_bias)."""
    Wq_w = np.asarray(Wq_w, np.float64)
    Wk_w = np.asarray(Wk_w, np.float64)
    Wq_b = np.asarray(Wq_b, np.float64)
    Wk_b = np.asarray(Wk_b, np.float64)
    D = Wq_w.shape[1]
    G = Wq_w.T @ Wk_w
    use_bias = bool(np.any(Wq_b) or np.any(Wk_b))
    if not use_bias:
        assert D % 128 == 0
        return G.astype(np.float32), D, False
    d_pad = ((D + 1 + 127) // 128) * 128
    Gp = np.zeros((d_pad, d_pad), np.float64)
    Gp[:D, :D] = G
    Gp[:D, D] = Wq_w.T @ Wk_b
    Gp[D, :D] = Wk_w.T @ Wq_b
    Gp[D, D] = Wq_b @ Wk_b
    return Gp.astype(np.float32), d_pad, True


def tile_pe(pe_core, NB, m):
    """[64, ns, m] f32 slice -> [64, NB, 16, 2, 2, 2, m] bf16 DMA image.

    n = nb*128 + t*8 + q*2 + s with q = 2*cc + qq; SBUF partition 2e + s,
    SBUF column q*m + mcol.  DRAM order (e, nb, t, cc, qq, s, m) makes each
    DMA piece (h, cc) read 32 contiguous 16 KiB chunks."""
    pb = np.asarray(pe_core, NPBF16)
    arr = pb.reshape(EMB, NB, 16, 2, 2, 2, m).transpose(0, 1, 2, 3, 5, 4, 6)
    return np.ascontiguousarray(arr).reshape(EMB, NB, 16, 2, 2, 2,
                                             m).view(np.uint32)


def prep_in_maps(ref_feat, sup_feat, position_embedding,
                 Wg_w, Wg_b, Wq_w, Wq_b, Wk_w, Wk_b, ncores):
    """Host-side layout prep + per-core sharding. Returns (in_maps, meta)."""
    ref_feat = np.asarray(ref_feat, np.float32)
    sup_feat = np.asarray(sup_feat, np.float32)
    pe = np.asarray(position_embedding, np.float32)
    N, D = ref_feat.shape
    M = sup_feat.shape[0]
    assert pe.shape == (1, EMB, N, M)
    assert N % ncores == 0
    ns = N // ncores
    NB = ns // 128

    G, d_pad, use_bias = make_gmat(Wq_w, Wq_b, Wk_w, Wk_b)
    dc = d_pad // 128
    MS = M // 512
    refT = np.zeros((d_pad, N), np.float32)
    refT[:D, :] = ref_feat.T
    supT = np.zeros((d_pad, M), np.float32)
    supT[:D, :] = sup_feat.T
    if use_bias:
        refT[D, :] = 1.0
        supT[D, :] = 1.0
    # SBUF images: [128, k*W + c] = logical row k*128+p, col c
    G_img = np.ascontiguousarray(
        G.reshape(dc, 128, d_pad).transpose(1, 0, 2).reshape(128, dc * d_pad))
    sup_img = np.ascontiguousarray(
        supT.reshape(dc, 128, MS, 512).transpose(2, 1, 0, 3)
        .reshape(MS, 128, dc * 512))
    W2, dupM, E32, brep = make_consts(Wg_w, Wg_b)

    in_maps = []
    for c in range(ncores):
        in_maps.append({
            "pe": tile_pe(pe[0, :, c * ns:(c + 1) * ns, :], NB, M),
            "refT": np.ascontiguousarray(
                refT[:, c * ns:(c + 1) * ns].reshape(dc, 128, ns)
                .transpose(1, 0, 2).reshape(128, dc * ns)),
            "supT": sup_img,
            "G": G_img,
            "W2": W2.astype(NPBF16),
            "dupM": dupM,
            "E32": E32,
            "brep": brep,
        })
    meta = dict(ns=ns, m=M, dc=d_pad // 128, scale=1.0 / math.sqrt(float(D)))
    return in_maps, meta


_module_cache = {}

KERNEL_OPTS = dict(pe_bufs=3)


def get_module(ns, m, dc, scale, **opts):
    kw = {**KERNEL_OPTS, **opts}
    key = (ns, m, dc, round(scale, 12), tuple(sorted(kw.items())))
    if key not in _module_cache:
        _module_cache[key] = build_module(ns, m, dc, scale, **kw)
    return _module_cache[key]


def refine(S, adj, ref_feat, sup_feat, position_embedding,
           Wg_w, Wg_b, Wq_w, Wq_b, Wk_w, Wk_b):
    """Exactly re-evaluate (f64) every entry with |S - 8| < TAU in place."""
    ii, jj = np.nonzero(np.abs(S - 8.0) < TAU)
    if ii.size == 0:
        return 0
    ui, inv_i = np.unique(ii, return_inverse=True)
    uj, inv_j = np.unique(jj, return_inverse=True)
    q = (np.asarray(ref_feat, np.float64)[ui] @
         np.asarray(Wq_w, np.float64).T + np.asarray(Wq_b, np.float64))
    k = (np.asarray(sup_feat, np.float64)[uj] @
         np.asarray(Wk_w, np.float64).T + np.asarray(Wk_b, np.float64))
    aff = np.einsum('nd,nd->n', q[inv_i], k[inv_j]) / math.sqrt(q.shape[1])
    pe_cols = np.asarray(position_embedding, np.float64)[0][:, ii, jj]
    w = np.maximum(np.asarray(Wg_w, np.float64) @ pe_cols
                   + np.asarray(Wg_b, np.float64)[:, None], 0.0)
    z = np.log(w + EPS) + aff[None, :]
    sig = (1.0 / (1.0 + np.exp(-z))).mean(axis=0)
    adj[ii, jj] = (sig > 0.5).astype(np.float32)
    return int(ii.size)


def kernel(ref_feat, sup_feat, position_embedding,
           Wg_w, Wg_b, Wq_w, Wq_b, Wk_w, Wk_b):
    from concourse.bass_utils import run_bass_kernel_spmd
    in_maps, meta = prep_in_maps(ref_feat, sup_feat, position_embedding,
                                 Wg_w, Wg_b, Wq_w, Wq_b, Wk_w, Wk_b, NCORES)
    nc = get_module(**meta)
    res = run_bass_kernel_spmd(nc, in_maps, core_ids=list(range(NCORES)))
    S = np.concatenate([r["S"] for r in res.results], axis=0)
    adj = (S < 8.0).astype(np.float32)
    refine(S, adj, ref_feat, sup_feat, position_embedding,
           Wg_w, Wg_b, Wq_w, Wq_b, Wk_w, Wk_b)
    return np.ascontiguousarray(adj, np.float32)
